# revision 1
# baseline (speedup 1.0000x reference)
"""Trainium2 Bass kernel for a 2-layer GRU + BN + FC head model.

Strategy (data-parallel over batch, 8 cores, per sharding hint):
  - Each core handles B_local = 16 of the 128 batch rows. Weights replicated.
  - Phase 1: xg0 = x @ W_ih0.T + (b_ih0 [+ b_hh0 for r,z gates]) as one big
    matmul over all 4096 local tokens, fp32r (full-rate PE), written to DRAM.
  - Phase 2: layer-0 sequential scan, 256 steps. Hidden matmul is
    weight-stationary: out[3H-tile(128), B(16)] = W_hh.T-tile.T @ hT-tile,
    with W_hh in bf16 (fast weight load) and the h state kept transposed
    [H-part, B-free] so gate math runs on all 128 partitions and h_new
    comes out already transposed. h0 stream written to DRAM.
  - Phase 3: xg1 = h0 @ W_ih1.T + bias, same big-matmul shape, fp32r.
  - Phase 4: layer-1 scan (no h stream output).
  - Phase 5: head on the final h state: BatchNorm (folded scale/bias) ->
    fc1+ReLU -> LayerNorm (via PE transpose to [B,256] row layout) -> fc2.
  - Output per core: outT [3, 16]; host reassembles [128, 3].
"""

import sys
from contextlib import ExitStack

import numpy as np

sys.path.insert(0, "/opt/trn_rl_repo")

import ml_dtypes  # noqa: E402
import concourse.bass as bass  # noqa: E402
import concourse.bacc as bacc  # noqa: E402
import concourse.tile as tile  # noqa: E402
from concourse import mybir  # noqa: E402
from concourse.bass import ds  # noqa: E402
from concourse.bass_utils import run_bass_kernel_spmd  # noqa: E402
from concourse.masks import make_identity  # noqa: E402

F32 = mybir.dt.float32
F32R = mybir.dt.float32r
BF16 = mybir.dt.bfloat16
AF = mybir.ActivationFunctionType
ALU = mybir.AluOpType

B, T, INP, H, OUT = 128, 256, 300, 512, 3
NCORES = 8
BL = B // NCORES            # 16 batch rows per core
TOK = BL * T                # 4096 local tokens
G = 3 * H                   # 1536 gate rows
MT = G // 128               # 12 gate tiles
KH = H // 128               # 4 hidden k-tiles
KI = 3                      # ceil(300/128) -> padded to 384
H2 = H // 2                 # 256
EPS = 1e-5
CH = 512                    # moving chunk (tokens) for projections
NCH = TOK // CH             # 8 chunks
SCAN_UNROLL = 8

_CACHE = {}


def _build_nc(reps=1, bench=False, phases=(1, 1, 1, 1)):
    nc = bacc.Bacc("TRN2", target_bir_lowering=False, debug=False)

    if bench:
        # benchmark variant: weights/x are internal (garbage) DRAM so each
        # run ships ~nothing over the wire; timing is value-independent
        def declare(name, shape, dtype, isOutput):
            if isOutput:
                return nc.declare_dram_parameter(name, shape, dtype, isOutput=True)
            return nc.dram_tensor(name, shape, dtype)
        nc.declare_dram_parameter("bench_in", [1, 1], F32, isOutput=False)
    else:
        declare = nc.declare_dram_parameter

    # ---- parameters (inputs) ----
    xT_p = declare("xT", [128, KI, TOK], F32R, isOutput=False)
    wih0_p = declare("wih0", [128, KI, G], F32R, isOutput=False)
    whh0_p = declare("whh0", [128, KH, G], BF16, isOutput=False)
    bias0_p = declare("bias0", [128, MT], F32, isOutput=False)
    bhhn0_p = declare("bhhn0", [128, KH], F32, isOutput=False)
    wih1_p = declare("wih1", [128, KH, G], BF16, isOutput=False)
    whh1_p = declare("whh1", [128, KH, G], BF16, isOutput=False)
    bias1_p = declare("bias1", [128, MT], F32, isOutput=False)
    bhhn1_p = declare("bhhn1", [128, KH], F32, isOutput=False)
    bnsc_p = declare("bnsc", [128, KH], F32, isOutput=False)
    bnbi_p = declare("bnbi", [128, KH], F32, isOutput=False)
    fc1w_p = declare("fc1w", [128, KH, H2], F32, isOutput=False)
    fc1b_p = declare("fc1b", [128, 2], F32, isOutput=False)
    lnw_p = declare("lnw", [H2], F32, isOutput=False)
    lnb_p = declare("lnb", [H2], F32, isOutput=False)
    fc2w_p = declare("fc2w", [128, 2, OUT], F32, isOutput=False)
    fc2b_p = declare("fc2b", [OUT, 1], F32, isOutput=False)
    outT_p = nc.declare_dram_parameter("outT", [OUT, BL], F32, isOutput=True)

    # ---- internal DRAM ----
    xg0_d = nc.dram_tensor("xg0_d", [128, T * MT * BL], F32)
    xg1_d = nc.dram_tensor("xg1_d", [128, T * MT * BL], F32)

    with tile.TileContext(nc) as tc, ExitStack() as ctx:
        cpool = ctx.enter_context(tc.tile_pool(name="const", bufs=1))
        mvpool = ctx.enter_context(tc.tile_pool(name="mv", bufs=1))
        wpool = ctx.enter_context(tc.tile_pool(name="work", bufs=3))
        ppool = ctx.enter_context(tc.tile_pool(name="proj_ps", bufs=3, space="PSUM"))
        spp = ctx.enter_context(tc.tile_pool(name="scan_ps", bufs=2, space="PSUM"))
        hpp = ctx.enter_context(tc.tile_pool(name="head_ps", bufs=1, space="PSUM"))
        spool = ctx.enter_context(tc.tile_pool(name="scan", bufs=4))
        stpool = ctx.enter_context(tc.tile_pool(name="state", bufs=1))

        # ---- persistent constants into SBUF ----
        def load_ktiles(p, k_n, width, dt, tag):
            t_ = cpool.tile([128, k_n, width], dt, tag=tag)
            nc.sync.dma_start(out=t_, in_=p[:])
            return t_

        wih0_sb = load_ktiles(wih0_p, KI, G, F32R, "wih0")
        whh0_sb = load_ktiles(whh0_p, KH, G, BF16, "whh0")
        wih1_sb = load_ktiles(wih1_p, KH, G, BF16, "wih1")
        whh1_sb = load_ktiles(whh1_p, KH, G, BF16, "whh1")
        fc1w_sb = load_ktiles(fc1w_p, KH, H2, F32, "fc1w")
        fc2w_sb = load_ktiles(fc2w_p, 2, OUT, F32, "fc2w")

        def load2d(p, shape, tag):
            t_ = cpool.tile(shape, F32, tag=tag)
            nc.sync.dma_start(out=t_, in_=p[:])
            return t_

        bias0_sb = load2d(bias0_p, [128, MT], "bias0")
        bhhn0_sb = load2d(bhhn0_p, [128, KH], "bhhn0")
        bias1_sb = load2d(bias1_p, [128, MT], "bias1")
        bhhn1_sb = load2d(bhhn1_p, [128, KH], "bhhn1")
        bnsc_sb = load2d(bnsc_p, [128, KH], "bnsc")
        bnbi_sb = load2d(bnbi_p, [128, KH], "bnbi")
        fc1b_sb = load2d(fc1b_p, [128, 2], "fc1b")
        fc2b_sb = load2d(fc2b_p, [OUT, 1], "fc2b")

        # ln_w/ln_b broadcast along partitions -> [BL, H2]
        def bcast(p, tag):
            t_ = cpool.tile([BL, H2], F32, tag=tag)
            src = p[:]
            bc = bass.AP(tensor=src.tensor, offset=src.offset,
                         ap=[[0, BL]] + list(src.ap))
            nc.sync.dma_start(out=t_, in_=bc)
            return t_

        lnw_sb = bcast(lnw_p, "lnw")
        lnb_sb = bcast(lnb_p, "lnb")

        hist_sb = cpool.tile([128, KH, TOK], BF16, tag="hist")
        ident_sb = cpool.tile([128, 128], F32, tag="ident")
        make_identity(nc, ident_sb)
        eps_sb = cpool.tile([128, 1], F32, tag="eps")
        nc.vector.memset(eps_sb, EPS)
        # warm-up per engine: absorb preamble waits so later real ops
        # don't exceed the per-instruction sync-wait limit
        warm = cpool.tile([128, 1], F32, tag="warm")
        nc.vector.memset(warm, 0.0)
        nc.scalar.copy(warm, warm)
        warm_ps = hpp.tile([1, 1], F32, tag="warm_ps")
        nc.tensor.matmul(warm_ps, warm, warm, start=True, stop=True)

        # ---- projection phase: dst[p, t*MT*BL + m*BL + b] = (W.T @ src)[.] + bias
        def projection(lhsT_sb, k_n, src_d, src_sb, dst_d, bias_sb):
            dst4 = dst_d[:].rearrange("p (t m b) -> p t m b", m=MT, b=BL)
            tpc = CH // BL  # 32 timesteps per chunk
            for half in range(2):
                if src_sb is None:
                    mv = []
                    for k in range(k_n):
                        mt = mvpool.tile([128, TOK // 2], F32R, tag=f"mv{k}")
                        nc.sync.dma_start(
                            out=mt,
                            in_=src_d[:, k, half * (TOK // 2):(half + 1) * (TOK // 2)])
                        mv.append(mt)
                for chn in range(NCH // 2):
                    c = half * (NCH // 2) + chn
                    for m in range(MT):
                        ps = ppool.tile([128, CH], F32, tag="proj")
                        for k in range(k_n):
                            if src_sb is None:
                                rhs = mv[k][:, chn * CH:(chn + 1) * CH]
                            else:
                                rhs = src_sb[:, k, c * CH:(c + 1) * CH]
                            nc.tensor.matmul(
                                ps, lhsT_sb[:, k, m * 128:(m + 1) * 128], rhs,
                                start=(k == 0), stop=(k == k_n - 1))
                        xo = wpool.tile([128, CH], F32, tag="proj_out")
                        nc.vector.tensor_scalar_add(xo, ps, bias_sb[:, m:m + 1])
                        nc.sync.dma_start(
                            out=dst4[:, c * tpc:(c + 1) * tpc, m, :],
                            in_=xo[:].rearrange("p (t b) -> p t b", b=BL))

        # ---- scan phase ----
        h_f32 = stpool.tile([128, KH, BL], F32, tag="h_f32")
        h_bf = stpool.tile([128, KH, BL], BF16, tag="h_bf")

        def scan(xg_d, whh_sb, bhhn_sb, write_h0, dma_eng=None):
            dma_eng = dma_eng or nc.sync
            nc.vector.memset(h_f32, 0.0)
            nc.vector.memset(h_bf, 0.0)
            xg4 = xg_d[:]


            def body(t):
                xg_t = spool.tile([128, MT, BL], F32, tag="xg_t")
                dma_eng.dma_start(
                    out=xg_t[:].rearrange("p m b -> p (m b)"),
                    in_=xg4[:, ds(t * (MT * BL), MT * BL)])
                hg = spp.tile([128, MT, BL], F32, tag="hg")
                for m in range(MT):
                    for k in range(KH):
                        nc.tensor.matmul(
                            hg[:, m, :], whh_sb[:, k, m * 128:(m + 1) * 128],
                            h_bf[:, k, :], start=(k == 0), stop=(k == KH - 1))
                rz = spool.tile([128, 8, BL], F32, tag="rz")
                nc.vector.tensor_add(rz, xg_t[:, 0:8, :], hg[:, 0:8, :])
                nc.scalar.activation(rz, rz, AF.Sigmoid)
                hn = spool.tile([128, KH, BL], F32, tag="hn")
                for k in range(KH):
                    # (hg_n + b_hh_n) * r
                    nc.vector.scalar_tensor_tensor(
                        hn[:, k, :], hg[:, 8 + k, :], bhhn_sb[:, k:k + 1],
                        rz[:, k, :], op0=ALU.add, op1=ALU.mult)
                nc.vector.tensor_add(hn, hn, xg_t[:, 8:12, :])
                nc.scalar.activation(hn, hn, AF.Tanh)
                d_ = spool.tile([128, KH, BL], F32, tag="d_")
                nc.vector.tensor_sub(d_, h_f32, hn)
                nc.vector.tensor_mul(d_, rz[:, 4:8, :], d_)
                nc.vector.tensor_add(h_f32, hn, d_)
                nc.vector.tensor_copy(h_bf, h_f32)
                if write_h0:
                    nc.vector.tensor_copy(hist_sb[:, :, ds(t * BL, BL)], h_bf)

            tc.For_i_unrolled(0, T, 1, body, max_unroll=SCAN_UNROLL)

        # ---- run the five phases (reps>1 only for benchmarking) ----
        for _rep in range(reps):
            if phases[0]:
                projection(wih0_sb, KI, xT_p, None, xg0_d, bias0_sb)
            if phases[1]:
                scan(xg0_d, whh0_sb, bhhn0_sb, write_h0=True)
            if phases[2]:
                projection(wih1_sb, KH, None, hist_sb, xg1_d, bias1_sb)
            if phases[3]:
                scan(xg1_d, whh1_sb, bhhn1_sb, write_h0=False, dma_eng=nc.scalar)

        # ---- head ----
        yT = wpool.tile([128, KH, BL], F32, tag="yT")
        for k in range(KH):
            nc.scalar.activation(yT[:, k, :], h_f32[:, k, :], AF.Identity,
                                 bias=bnbi_sb[:, k:k + 1], scale=bnsc_sb[:, k:k + 1])
        ps1 = hpp.tile([128, 2, BL], F32, tag="head")
        for m in range(2):
            for k in range(KH):
                nc.tensor.matmul(ps1[:, m, :], fc1w_sb[:, k, m * 128:(m + 1) * 128],
                                 yT[:, k, :], start=(k == 0), stop=(k == KH - 1))
        r1 = wpool.tile([128, 2, BL], F32, tag="r1")
        for m in range(2):
            nc.scalar.activation(r1[:, m, :], ps1[:, m, :], AF.Relu,
                                 bias=fc1b_sb[:, m:m + 1])
        pt = hpp.tile([BL, 2, 128], F32, tag="head")
        for m in range(2):
            nc.tensor.transpose(pt[:, m, :], r1[:, m, :], ident_sb)
        x1 = wpool.tile([BL, 2 * 128], F32, tag="x1")
        nc.vector.tensor_copy(x1, pt[:].rearrange("p m c -> p (m c)"))
        stats = wpool.tile([BL, 6], F32, tag="st")
        nc.vector.bn_stats(stats, x1)
        mv_ = wpool.tile([BL, 2], F32, tag="mv_")
        nc.vector.bn_aggr(mv_, stats)
        std = wpool.tile([BL, 1], F32, tag="std")
        nc.scalar.activation(std, mv_[:, 1:2], AF.Sqrt, bias=eps_sb[:BL, :])
        rstd = wpool.tile([BL, 1], F32, tag="rstd")
        nc.vector.reciprocal(rstd, std)
        nmu = wpool.tile([BL, 1], F32, tag="nmu")
        nc.vector.scalar_tensor_tensor(nmu, mv_[:, 0:1], -1.0, rstd,
                                       op0=ALU.mult, op1=ALU.mult)
        xn = wpool.tile([BL, 2 * 128], F32, tag="xn")
        nc.scalar.activation(xn, x1, AF.Identity, bias=nmu, scale=rstd)
        nc.vector.tensor_mul(xn, xn, lnw_sb)
        nc.vector.tensor_add(xn, xn, lnb_sb)
        ptb = hpp.tile([128, 2, BL], F32, tag="head")
        for m in range(2):
            nc.tensor.transpose(ptb[:, m, :], xn[:, m * 128:(m + 1) * 128],
                                ident_sb[:BL, :BL])
        xnT = wpool.tile([128, 2, BL], F32, tag="xnT")
        nc.vector.tensor_copy(xnT, ptb)
        ps2 = hpp.tile([OUT, BL], F32, tag="head")
        for k in range(2):
            nc.tensor.matmul(ps2, fc2w_sb[:, k, :], xnT[:, k, :],
                             start=(k == 0), stop=(k == 1))
        oT = wpool.tile([OUT, BL], F32, tag="oT")
        nc.scalar.activation(oT, ps2, AF.Identity, bias=fc2b_sb[:])
        nc.sync.dma_start(out=outT_p[:], in_=oT)

    nc.compile()
    return nc


def _to_f32(a):
    return np.ascontiguousarray(np.asarray(a, dtype=np.float32))


def _prep_core_inputs(inputs, c):
    """Build the in_map for core c from the full inputs."""
    x = _to_f32(inputs["x"])                      # [B, T, INP]
    xc = x[c * BL:(c + 1) * BL]                   # [BL, T, INP]
    xT = np.zeros((KI * 128, TOK), np.float32)
    xT[:INP] = xc.transpose(2, 1, 0).reshape(INP, TOK)

    def ktiles(wT, k_n, width):
        out = np.zeros((k_n * 128, width), np.float32)
        out[:wT.shape[0]] = wT
        return np.ascontiguousarray(
            out.reshape(k_n, 128, width).transpose(1, 0, 2))

    m = {}
    m["xT"] = np.ascontiguousarray(xT.reshape(KI, 128, TOK).transpose(1, 0, 2))
    for layer in range(2):
        w_ih = _to_f32(inputs[f"w_ih_l{layer}"])  # [G, in]
        w_hh = _to_f32(inputs[f"w_hh_l{layer}"])  # [G, H]
        b_ih = _to_f32(inputs[f"b_ih_l{layer}"])
        b_hh = _to_f32(inputs[f"b_hh_l{layer}"])
        k_n = KI if layer == 0 else KH
        wihT = ktiles(w_ih.T, k_n, G)
        m[f"wih{layer}"] = wihT.astype(ml_dtypes.bfloat16) if layer == 1 else wihT
        m[f"whh{layer}"] = ktiles(w_hh.T, KH, G).astype(ml_dtypes.bfloat16)
        bias = b_ih.copy()
        bias[:2 * H] += b_hh[:2 * H]
        m[f"bias{layer}"] = np.ascontiguousarray(bias.reshape(MT, 128).T)
        m[f"bhhn{layer}"] = np.ascontiguousarray(b_hh[2 * H:].reshape(KH, 128).T)
    bn_sc = _to_f32(inputs["bn_w"]) / np.sqrt(_to_f32(inputs["bn_var"]) + EPS)
    bn_bi = _to_f32(inputs["bn_b"]) - _to_f32(inputs["bn_mean"]) * bn_sc
    m["bnsc"] = np.ascontiguousarray(bn_sc.reshape(KH, 128).T)
    m["bnbi"] = np.ascontiguousarray(bn_bi.reshape(KH, 128).T)
    m["fc1w"] = ktiles(_to_f32(inputs["fc1_w"]).T, KH, H2)
    m["fc1b"] = np.ascontiguousarray(_to_f32(inputs["fc1_b"]).reshape(2, 128).T)
    m["lnw"] = _to_f32(inputs["ln_w"])
    m["lnb"] = _to_f32(inputs["ln_b"])
    m["fc2w"] = ktiles(_to_f32(inputs["fc2_w"]).T, 2, OUT)
    m["fc2b"] = _to_f32(inputs["fc2_b"]).reshape(OUT, 1)
    return m


def _run(inputs, trace=False):
    if "nc" not in _CACHE:
        _CACHE["nc"] = _build_nc()
    nc = _CACHE["nc"]
    in_maps = [_prep_core_inputs(inputs, c) for c in range(NCORES)]
    res = run_bass_kernel_spmd(nc, in_maps, list(range(NCORES)), trace=trace)
    out = np.empty((B, OUT), np.float32)
    for c in range(NCORES):
        out[c * BL:(c + 1) * BL] = np.asarray(res.results[c]["outT"]).T
    return out, res


def kernel(**inputs):
    out, _ = _run(inputs)
    return out

